# revision 31
# baseline (speedup 1.0000x reference)
"""Trainium2 Bass kernel for nn_CrossAttention (3x3 scale-grid cross attention).

Reference computation (per batch b):
    WV_i = V_i @ W.T + b                    (video projection, i in 0..2)
    S_ij = (WV_i @ A_j.T) / sqrt(C)         [T, S] scores
    P_ij = softmax(S_ij, axis=-1)
    fv[i,j] = P_ij @ A_j        -> out[0, i, j, b]
    fa[j,i] = P_ij.T @ V_i      -> out[1, j, i, b]

Sharding: data-parallel over batch B=8 across the 8 NeuronCores (one batch
element per core). W/b replicated. Each core runs all 9 (i,j) attention pairs
for its batch element.

On-chip plan (per core): bf16 matmul paths with fp32 PSUM accumulation.
Softmax normalization is folded into the outputs (fv scaled at the PSUM->SBUF
copy, fa via a row-scaled copy of V). The host pre-transposes A/V/W (layout
prep only). P^T is produced by XBAR DMA transposes (SP HWDGE queue) instead
of PE identity-matmul transposes, freeing the tensor engine (~31us) and two
PSUM banks; PTa is double-buffered across pairs so transposes never wait on
the previous pair's fv reads. Outputs are staged to bf16 in SBUF and written
as half-pair DMAs (quarter-sized at the kernel tail), halving output HBM
traffic and cutting descriptor count ~4x. Inputs load as one merged DMA per
tensor in consumption order. fv blocks run one ahead of fa blocks so the PE
never stalls on the tail exp->recip->Vr chain at pair boundaries.

Cost-model timeline: 400.7us vs 434.4us for the PE-transpose baseline; PE
busy 96.7% at its 389us bf16 GEMM floor. fp8 paths were evaluated and
rejected: e4m3 quantization of P/scores/rhs exceeds the 2e-2 tolerance
(max err 1.8e-2..5e-2 measured on real inputs).
"""

import numpy as np
from contextlib import ExitStack

import ml_dtypes

import concourse.bacc as bacc
import concourse.bass as bass
import concourse.mybir as mybir
import concourse.tile as tile
from concourse.bass_utils import run_bass_kernel_spmd

BF16 = mybir.dt.bfloat16
F32 = mybir.dt.float32
AF = mybir.ActivationFunctionType

B, T, C = 8, 1024, 512
P = 128
NT = T // P   # 8 row blocks
ND = C // P   # 4 feature chunks
SCALE = 1.0 / float(np.sqrt(C))

_CACHE = {}


def _build(repeat=1):
    key = ("nc", repeat)
    if key in _CACHE:
        return _CACHE[key]

    nc = bacc.Bacc("TRN2", target_bir_lowering=False, debug=False, num_devices=8)

    at_dram = [nc.dram_tensor(f"at{j}", [C, T], BF16, kind="ExternalInput").ap()
               for j in range(3)]
    vt_dram = [nc.dram_tensor(f"vt{i}", [C, T], BF16, kind="ExternalInput").ap()
               for i in range(3)]
    a_dram = [nc.dram_tensor(f"a{j}", [T, C], BF16, kind="ExternalInput").ap()
              for j in range(3)]
    v_dram = [nc.dram_tensor(f"v{i}", [T, C], BF16, kind="ExternalInput").ap()
              for i in range(3)]
    wt_dram = nc.dram_tensor("WT", [C, C], BF16, kind="ExternalInput").ap()
    b_dram = nc.dram_tensor("bvec", [ND, P, 1], F32, kind="ExternalInput").ap()
    out_dram = nc.dram_tensor("out", [2, 3, 3, T, C], BF16,
                              kind="ExternalOutput").ap()

    with ExitStack() as ctx:
        tc = ctx.enter_context(tile.TileContext(nc))

        const = ctx.enter_context(tc.tile_pool(name="const", bufs=1))
        big = ctx.enter_context(tc.tile_pool(name="big", bufs=1))
        work = ctx.enter_context(tc.tile_pool(name="work", bufs=1))
        small = ctx.enter_context(tc.tile_pool(name="small", bufs=1))
        stage = ctx.enter_context(tc.tile_pool(name="stage", bufs=1))
        pta = ctx.enter_context(tc.tile_pool(name="pta", bufs=2))

        ps_s = ctx.enter_context(tc.tile_pool(name="ps_s", bufs=2, space="PSUM"))
        ps_o = ctx.enter_context(tc.tile_pool(name="ps_o", bufs=4, space="PSUM"))

        for _rep in range(repeat):
            _kernel_body(nc, tc, const, big, work, small, stage, pta,
                         ps_s, ps_o,
                         a_dram, v_dram, at_dram, vt_dram, wt_dram, b_dram,
                         out_dram)

    nc.compile()
    _CACHE[key] = nc
    return nc


def _kernel_body(nc, tc, const, big, work, small, stage, pta,
                 ps_s, ps_o,
                 a_dram, v_dram, at_dram, vt_dram, wt_dram, b_dram, out_dram):
    # ---- load operands (bf16; transposed copies prepared on host) ----
    # Merged tiles, one DMA per tensor. Issue order = startup critical path:
    # the first WV matmul needs WT plus VT[0].
    WT = big.tile([P, ND, C], BF16, tag="WT", name="WT")
    VT = [big.tile([P, ND, T], BF16, tag=f"VT{i}", name=f"VT{i}")
          for i in range(3)]
    AT = [big.tile([P, ND, T], BF16, tag=f"AT{j}", name=f"AT{j}")
          for j in range(3)]
    # WT and VT0 split per-cc (SP / ACT queues) so the first WV matmul
    # (needs only the cc=0 slices) starts as early as possible.
    vt0 = vt_dram[0].rearrange("(cc p) t -> p cc t", cc=ND)
    wtr = wt_dram.rearrange("(cc p) d -> p cc d", cc=ND)
    for cc in range(ND):
        nc.sync.dma_start(WT[:, cc], wtr[:, cc])
        nc.scalar.dma_start(VT[0][:, cc], vt0[:, cc])

    Abf = [big.tile([P, NT, C], BF16, tag=f"A{j}", name=f"A{j}")
           for j in range(3)]
    Vbf = [big.tile([P, NT, C], BF16, tag=f"V{i}", name=f"V{i}")
           for i in range(3)]

    # remaining loads in consumption order: scores(0,0) needs AT0, the first
    # Vr needs V0, fv(0,0) needs A0; later pairs consume AT1/2, A1/2, V1/2.
    b_sb = []
    for dc in range(ND):
        t_ = const.tile([P, 1], F32, tag=f"b{dc}", name=f"b{dc}")
        nc.sync.dma_start(t_[:], b_dram[dc])
        b_sb.append(t_)

    def _load_at(j, eng):
        eng.dma_start(AT[j][:],
                      at_dram[j].rearrange("(cc p) t -> p cc t", cc=ND))

    def _load_a(j, eng):
        eng.dma_start(Abf[j][:],
                      a_dram[j].rearrange("(tb p) c -> p tb c", tb=NT))

    def _load_v(i, eng):
        eng.dma_start(Vbf[i][:],
                      v_dram[i].rearrange("(tb p) c -> p tb c", tb=NT))

    # single queue, consumption order: VT1/VT2 feed the WV phase (first
    # ~20us), then AT0 (scores at ~23us), V0 (first Vr at ~26us), A0
    # (fv(0,0) at ~45us), then the later pairs' operands.
    for i in range(1, 3):
        nc.sync.dma_start(VT[i][:],
                          vt_dram[i].rearrange("(cc p) t -> p cc t", cc=ND))
    _load_at(0, nc.sync)
    _load_v(0, nc.sync)
    _load_a(0, nc.sync)
    _load_at(1, nc.sync)
    _load_at(2, nc.sync)
    _load_a(1, nc.sync)
    _load_a(2, nc.sync)
    _load_v(1, nc.sync)
    _load_v(2, nc.sync)

    # ---- WV^T_i[d, t] = W^T @ V^T_i + b (bf16 out, bias folded in) ----
    # cc outer / th inner: consecutive matmuls share the stationary operand,
    # halving LDWEIGHTS traffic (the two halves accumulate in two banks).
    WVT = [big.tile([P, ND, T], BF16, tag=f"WVT{i}", name=f"WVT{i}")
           for i in range(3)]

    def emit_wv(i):
        for dc in range(ND):
            po2 = [ps_o.tile([P, C], F32, tag="o", name="o") for _ in range(2)]
            for cc in range(ND):
                for th in range(2):
                    nc.tensor.matmul(po2[th][:], WT[:, cc, dc * P:(dc + 1) * P],
                                     VT[i][:, cc, th * C:(th + 1) * C],
                                     start=(cc == 0), stop=(cc == ND - 1))
            for th in range(2):
                nc.scalar.activation(WVT[i][:, dc, th * C:(th + 1) * C],
                                     po2[th][:], AF.Identity,
                                     bias=b_sb[dc][:], scale=1.0)

    for i in range(3):
        emit_wv(i)

    # ---- main loop over the 9 attention pairs ----
    for i in range(3):
        for j in range(3):
            last_pair = (i == 2 and j == 2)
            Pt = [work.tile([P, T], BF16, tag=f"P{tb}", name=f"P{tb}")
                  for tb in range(NT)]
            # double-buffered across pairs: pair p+1's transposes need not
            # wait for pair p's fv reads (SP queue would head-of-line block)
            PTa = pta.tile([P, NT, T], BF16, tag="PTall", name="PTall")
            recip = [small.tile([P, 1], F32, tag=f"rc{tb}", name=f"rc{tb}")
                     for tb in range(NT)]
            Vr = [work.tile([P, C], BF16, tag=f"Vr{tb}", name=f"Vr{tb}")
                  for tb in range(NT)]

            for tb in range(NT):
                # one [128, 1024] score block = 2 PSUM banks; each matmul
                # stays within one bank.
                ps = ps_s.tile([P, T], F32, tag="s", name="s")
                for h in range(2):
                    for dc in range(ND):
                        nc.tensor.matmul(ps[:, h * C:(h + 1) * C],
                                         WVT[i][:, dc, tb * P:(tb + 1) * P],
                                         AT[j][:, dc, h * C:(h + 1) * C],
                                         start=(dc == 0), stop=(dc == ND - 1))
                rsum = small.tile([P, 1], F32, tag=f"rsum{tb}", name=f"rsum{tb}")
                nc.scalar.activation(Pt[tb][:], ps[:], AF.Exp, scale=SCALE,
                                     accum_out=rsum[:])
                nc.vector.reciprocal(recip[tb][:], rsum[:])
                nc.vector.tensor_scalar_mul(Vr[tb][:], Vbf[i][:, tb, :],
                                            recip[tb][:])
                # P^T via one XBAR DMA transpose (SP HWDGE queue):
                # PTa[sp, sc, tb*P+τ] = Pt[tb][τ, sc*P+sp]
                nc.sync.dma_start(PTa[:, :, tb * P:(tb + 1) * P],
                                  Pt[tb][:], transpose=True)

            # fa[j,i] = P_raw^T @ (diag(recip) @ V_i)
            # fv[i,j] = diag(recip) @ (P_raw @ A_j)
            # fv(k) depends only on the transposes of Pt[k] (long done for
            # small k); fa(k) needs all Vr. Run fv one block ahead of fa so
            # the PE never waits on the pair's tail exp/recip/Vr chain.
            # output staging in halves (k 0-3 / 4-7): the first store of each
            # stream overlaps the second half's matmuls, and the kernel tail
            # only waits on a 0.5 MB DMA. fv stores ride the ACT HWDGE queue,
            # fa stores the SP queue.
            H = NT // 2
            Sfv = [stage.tile([P, H, C], BF16, tag=f"sfv{h}", name=f"sfv{h}")
                   for h in range(2)]
            Sfa = [stage.tile([P, H, C], BF16, tag=f"sfa{h}", name=f"sfa{h}")
                   for h in range(2)]
            ofv = out_dram[0, i, j].rearrange("(h k p) c -> h p k c", h=2, k=H)
            ofa = out_dram[1, j, i].rearrange("(h k p) c -> h p k c", h=2, k=H)

            def fv_block(k):
                po = ps_o.tile([P, C], F32, tag="o", name="o")
                for sc in range(NT):
                    nc.tensor.matmul(po[:], PTa[:, sc, k * P:(k + 1) * P],
                                     Abf[j][:, sc, :],
                                     start=(sc == 0), stop=(sc == NT - 1))
                nc.scalar.activation(Sfv[k // H][:, k % H, :], po[:], AF.Copy,
                                     bias=0.0, scale=recip[k][:])
                if last_pair and k >= NT - 2:
                    # quarter stores at the very end shrink the kernel tail
                    nc.scalar.dma_start(ofv[k // H][:, k % H], Sfv[k // H][:, k % H])
                    if k == NT - 2:
                        nc.scalar.dma_start(ofv[1][:, 0:H - 2], Sfv[1][:, 0:H - 2])
                elif k % H == H - 1:
                    nc.scalar.dma_start(ofv[k // H], Sfv[k // H][:])

            def fa_block(k):
                po = ps_o.tile([P, C], F32, tag="o", name="o")
                for tb in range(NT):
                    nc.tensor.matmul(po[:], Pt[tb][:, k * P:(k + 1) * P],
                                     Vr[tb][:],
                                     start=(tb == 0), stop=(tb == NT - 1))
                nc.vector.tensor_copy(Sfa[k // H][:, k % H, :], po[:])
                if last_pair and k >= NT - 2:
                    nc.sync.dma_start(ofa[k // H][:, k % H], Sfa[k // H][:, k % H])
                    if k == NT - 2:
                        nc.sync.dma_start(ofa[1][:, 0:H - 2], Sfa[1][:, 0:H - 2])
                elif k % H == H - 1:
                    nc.sync.dma_start(ofa[k // H], Sfa[k // H][:])

            fv_block(0)
            fv_block(1)
            for k in range(NT - 2):
                fa_block(k)
                fv_block(k + 2)
            fa_block(NT - 2)
            fa_block(NT - 1)


def _prep_in_maps(a0, a1, a2, v0, v1, v2, W, b):
    bf = ml_dtypes.bfloat16
    a_bf = [np.asarray(x, dtype=np.float32).astype(bf) for x in (a0, a1, a2)]
    v_bf = [np.asarray(x, dtype=np.float32).astype(bf) for x in (v0, v1, v2)]
    wt_bf = np.ascontiguousarray(np.asarray(W, dtype=np.float32).astype(bf).T)
    b_r = np.ascontiguousarray(
        np.asarray(b, dtype=np.float32).reshape(ND, P, 1))
    in_maps = []
    for bi in range(B):
        m = {f"a{j}": np.ascontiguousarray(a_bf[j][bi]) for j in range(3)}
        m.update({f"v{i}": np.ascontiguousarray(v_bf[i][bi]) for i in range(3)})
        m.update({f"at{j}": np.ascontiguousarray(a_bf[j][bi].T)
                  for j in range(3)})
        m.update({f"vt{i}": np.ascontiguousarray(v_bf[i][bi].T)
                  for i in range(3)})
        m["WT"] = wt_bf
        m["bvec"] = b_r
        in_maps.append(m)
    return in_maps


def run(inputs, trace=False, tmpdir=None):
    """Build+run on 8 cores; returns (full_output, BassKernelResults)."""
    nc = _build()
    in_maps = _prep_in_maps(**inputs)
    res = run_bass_kernel_spmd(nc, in_maps, list(range(B)), trace=trace,
                               tmpdir=tmpdir)
    out = np.empty((2, 3, 3, B, T, C), dtype=np.float32)
    for bi in range(B):
        out[:, :, :, bi] = res.results[bi]["out"].astype(np.float32)
    return out, res


def kernel(a0, a1, a2, v0, v1, v2, W, b):
    out, _ = run(dict(a0=a0, a1=a1, a2=a2, v0=v0, v1=v1, v2=v2, W=W, b=b))
    return out
